# revision 6
# baseline (speedup 1.0000x reference)
"""DPConv (kernel=8, ext=4, stride=4) on 8 TRN2 NeuronCores — v10.

Math: with K = k + 2e = 16 and k = 8, DPConv collapses to
out = L @ img @ L.T per (n, c) image (L = exact 1-D operator, entries
n/16).  The column pass factors through natural pairs
P[s] = x[2s] + x[2s+1] (carrying the matmul's Lq = L/4 scaling):

    out[:, 4a+b] = P[:, 2a+b-2] + P[:, 2a+b]      (a = 1..30, b = 0..3)
    out[:, {0,1,126,127}] = T(x_col{0,127});  out[:, {2,3}] = 2 P[:, {0,1}]
    out[:, {124,125}] = 2 P[:, {62,63}]

On-chip structure:
  * pairsum rides the matmul: host stages each image [evens64 | odds64];
    two accumulating matmuls (start/stop) give PSUM = Lq@e + Lq@o = P.
  * ALL eight edge columns per image come from one 512-col "E" matmul:
    the host stages 8 pre-scaled edge columns per image
    [4x0, 4x0, 2(x0+x1), 2(x2+x3), 2(x124+x125), 2(x126+x127), 4x127,
    4x127] so psE2[:, i, s] IS the final edge-column value — one strided
    DVE copy per group writes out cols {0..3, 124..127}.
  * interior fold = two DVE adds with 4B-aligned 2-element runs (2x bf16
    mode); P evacuation PSUM->SBUF bf16 is ACT's only compute.

Engine roles (each queue has one job, so nothing parks behind a
load-dependent wait — the v8/v9 lesson):
  Sync:   load chunks [lt+g0, g2, g4, g6, g8]      (SP HWDGE ring)
  ACT:    load chunks [E, g1, g3, g5, g7] (issued first, high priority),
          then per-group P evacuation; the FINAL small store rides this
          ring too (HWDGE latency 0.6us vs SWDGE 1.2us, and the ring's
          loads are long done by then)
  PE:     matmuls (2 per group + E)
  DVE:    fold_even + fold_odd + edge copy
  GpSimd: store DMA issues via SWDGE — they wait only on their own
          producers, so store packets interleave with load packets in
          the shared SDMA engines from ~1/3 into the kernel.
Group sizes (4,8,8,8,8,8,8,8,4): small first group starts the pipeline
early, small last group shortens the landed->stored tail latency.

Sharding: pure data parallel — core k takes batch element n = k.
"""

import ml_dtypes
import numpy as np

import concourse.bacc as bacc
import concourse.mybir as mybir
import concourse.tile as tile
from concourse import bass_utils
from concourse.ap import AP

N_CORES = 8
C_PER_CORE = 64
GROUPS = (4, 8, 8, 8, 8, 8, 8, 8, 4)
STARTS = (0, 4, 12, 20, 28, 36, 44, 52, 60)
# dram column layout: [lt(128) | g0(512) | E(512) | g1..g7(1024 each) | g8(512)]
LT0, G0_0, E0 = 0, 128, 640
XB_COLS = 8832
F32 = mybir.dt.float32
BF16 = mybir.dt.bfloat16
BF16_NP = ml_dtypes.bfloat16
assert sum(GROUPS) == C_PER_CORE


def _gcol(g):
    """dram start column of group g's image block."""
    if g == 0:
        return G0_0
    return 1152 + (g - 1) * 1024


def _build_lq() -> np.ndarray:
    """The 1-D DPConv operator with the column-pass 1/4 folded in: L/4."""
    L = np.zeros((128, 128), np.float64)
    for w in range(128):
        i_lo = max(0, -((7 - w) // 4))      # ceil((w-7)/4)
        i_hi = min(30, w // 4)
        for i in (i_lo, i_hi):              # counted twice when equal
            L[w, min(127, max(0, 2 * w - 4 * i - 4))] += 0.25
            L[w, min(127, max(0, 2 * w - 4 * i - 3))] += 0.25
    return (L / 4.0).astype(np.float32)


_LQ_T = np.ascontiguousarray(_build_lq().T)          # lhsT layout [r, h]
_LQ_T_BF16 = _LQ_T.astype(BF16_NP)
assert np.all(_LQ_T_BF16.astype(np.float32) == _LQ_T)  # L exact in bf16


def _as_strided(base: AP, dims) -> AP:
    return AP(base.tensor, base.offset, dims)


def _flat(ap: AP, n: int) -> AP:
    pdim = list(ap.ap[0])
    return AP(ap.tensor, ap.offset, [pdim, [1, n]])


def _dpconv_tile(tc, o_d, xb_d):
    nc = tc.nc
    with tc.tile_pool(name="const", bufs=1) as cp, \
         tc.tile_pool(name="in", bufs=1) as inp, \
         tc.tile_pool(name="io", bufs=1) as iop, \
         tc.tile_pool(name="mid", bufs=4) as mp, \
         tc.tile_pool(name="ps", bufs=4, space="PSUM") as pp, \
         tc.tile_pool(name="psE", bufs=1, space="PSUM") as ppE:
        # ---- all load DMAs first, split across the two HWDGE rings ----
        with tc.high_priority():
            c0t = cp.tile([128, 640], BF16)            # lt + g0 images
            nc.sync.dma_start(out=c0t[:], in_=xb_d[:, 0:640])
            et = cp.tile([128, 512], BF16, tag="E")    # edge-column block
            nc.scalar.dma_start(out=et[:], in_=xb_d[:, E0:E0 + 512])
            img_tiles = {0: (c0t, G0_0)}
            for g in range(1, len(GROUPS)):
                cn = GROUPS[g]
                ct = inp.tile([128, cn, 128], BF16, tag=f"in{g}", name=f"ct{g}")
                assert list(ct[:].ap[1])[0] == 128
                eng = nc.sync if g % 2 == 0 else nc.scalar
                eng.dma_start(
                    out=_flat(ct[:], cn * 128),
                    in_=xb_d[:, _gcol(g):_gcol(g) + cn * 128])
                img_tiles[g] = (ct, 0)
        lt = c0t[:, 0:128]

        # edge matmul: psE2[:, i, s] = final out cols {0..3,124..127}
        psE = ppE.tile([128, C_PER_CORE, 8], F32)
        assert list(psE[:].ap[1])[0] == 8
        nc.tensor.matmul(psE[:], lt, et[:], start=True, stop=True)

        for g, (i0, G) in enumerate(zip(STARTS, GROUPS)):
            ct, cofs = img_tiles[g]

            def img_ap(i, lo, hi, n):
                off = ct[:].offset + cofs + i * 128 + lo
                return AP(ct[:].tensor, off,
                          [list(ct[:].ap[0]), [128, n], [1, hi - lo]])

            # pairsum-in-PSUM: P = Lq@evens + Lq@odds (accumulate)
            pt = pp.tile([128, 8, 64], F32, tag="P")
            assert list(pt[:].ap[1])[0] == 64
            po = pt[:, 0:G, :]
            nc.tensor.matmul(po, lt, img_ap(0, 0, 64, G),
                             start=True, stop=False)
            nc.tensor.matmul(po, lt, img_ap(0, 64, 128, G),
                             start=False, stop=True)

            # ACT: evacuate P to SBUF bf16
            ps = mp.tile([128, 8, 64], BF16, tag="P16")
            assert list(ps[:].ap[1])[0] == 64
            nc.scalar.copy(out=ps[:, 0:G, :], in_=pt[:, 0:G, :])

            ot = iop.tile([128, G, 128], BF16, tag=f"out{g}", name=f"ot{g}")
            assert list(ot[:].ap[1])[0] == 128
            pd = list(ps[:].ap[0])
            gdim = [64, G]
            od0 = list(ot[:].ap[0])
            ogdim = [128, G]

            # DVE interior fold: out[4a+b] = P[2a+b-2] + P[2a+b], a=1..30,
            # split by b-parity -> 4B-aligned 2-element bf16 runs (2x mode)
            nc.vector.tensor_add(
                out=_as_strided(ot[:, 0:1, 4:5], [od0, ogdim, [4, 30], [1, 2]]),
                in0=_as_strided(ps[:, 0:1, 0:1], [pd, gdim, [2, 30], [1, 2]]),
                in1=_as_strided(ps[:, 0:1, 2:3], [pd, gdim, [2, 30], [1, 2]]))
            nc.vector.tensor_add(
                out=_as_strided(ot[:, 0:1, 6:7], [od0, ogdim, [4, 30], [1, 2]]),
                in0=_as_strided(ps[:, 0:1, 2:3], [pd, gdim, [2, 30], [1, 2]]),
                in1=_as_strided(ps[:, 0:1, 4:5], [pd, gdim, [2, 30], [1, 2]]))

            # DVE edge copy: out cols {0..3,124..127} <- psE2 values
            pe = psE[:, i0:i0 + G, 0:1]
            nc.vector.tensor_copy(
                out=_as_strided(ot[:, 0:1, 0:1], [od0, ogdim, [124, 2], [1, 4]]),
                in_=_as_strided(pe, [list(pe.ap[0]), [8, G], [4, 2], [1, 4]]))

            # store: SWDGE for the pipeline body, ACT HWDGE ring for the
            # final small group (lower first-byte latency, ring idle then)
            st_eng = nc.scalar if g == len(GROUPS) - 1 else nc.gpsimd
            st_eng.dma_start(
                out=o_d[:, i0 * 128:(i0 + G) * 128],
                in_=_flat(ot[:], G * 128))


_CACHE = {}


def _get_nc():
    if "nc" not in _CACHE:
        nc = bacc.Bacc("TRN2", target_bir_lowering=False, debug=False)
        xb_d = nc.dram_tensor("xb", (128, XB_COLS), BF16,
                              kind="ExternalInput").ap()
        o_d = nc.dram_tensor("o", (128, C_PER_CORE * 128), BF16,
                             kind="ExternalOutput").ap()
        with tile.TileContext(nc) as tc:
            _dpconv_tile(tc, o_d, xb_d)
        nc.compile()
        _CACHE["nc"] = nc
    return _CACHE["nc"]


def _stage(xk: np.ndarray) -> np.ndarray:
    """[C,H,W] f32 -> [128, XB_COLS] bf16 in the v10 dram layout."""
    t = xk.transpose(1, 0, 2)                      # [H, C, W]
    out = np.empty((128, XB_COLS), np.float32)
    out[:, 0:128] = _LQ_T
    img = np.concatenate([t[:, :, 0::2], t[:, :, 1::2]], axis=2)  # [H,C,128]
    # edge block: 8 pre-scaled columns per image
    E = np.empty((128, C_PER_CORE, 8), np.float32)
    E[:, :, 0] = E[:, :, 1] = 4.0 * t[:, :, 0]
    E[:, :, 2] = 2.0 * (t[:, :, 0] + t[:, :, 1])
    E[:, :, 3] = 2.0 * (t[:, :, 2] + t[:, :, 3])
    E[:, :, 4] = 2.0 * (t[:, :, 124] + t[:, :, 125])
    E[:, :, 5] = 2.0 * (t[:, :, 126] + t[:, :, 127])
    E[:, :, 6] = E[:, :, 7] = 4.0 * t[:, :, 127]
    out[:, E0:E0 + 512] = E.reshape(128, 512)
    for g, (i0, G) in enumerate(zip(STARTS, GROUPS)):
        out[:, _gcol(g):_gcol(g) + G * 128] = \
            img[:, i0:i0 + G, :].reshape(128, G * 128)
    return out.astype(BF16_NP)


def run(x: np.ndarray, **spmd_kwargs) -> bass_utils.BassKernelResults:
    """Shard x (8,64,128,128) across 8 cores and run the Bass kernel."""
    nc = _get_nc()
    in_maps = [{"xb": _stage(x[k])} for k in range(N_CORES)]
    return bass_utils.run_bass_kernel_spmd(
        nc, in_maps, core_ids=list(range(N_CORES)), **spmd_kwargs)


def kernel(x) -> np.ndarray:
    x = np.asarray(x, dtype=np.float32)
    assert x.shape == (N_CORES, C_PER_CORE, 128, 128), x.shape
    res = run(x)
    return np.stack(
        [res.results[k]["o"].reshape(128, C_PER_CORE, 128)
         .astype(np.float32).transpose(1, 0, 2)
         for k in range(N_CORES)],
        axis=0)
